# revision 34
# baseline (speedup 1.0000x reference)
"""GQA causal attention block (B=2, L=2048, d_model=2048, 32 Q heads / 8 KV
heads) on 8 TRN2 NeuronCores. HW-verified: 471602 ns, rel err 6.6e-03
(baseline: 728273 ns).

Sharding: 8-way tensor parallel over heads. Core c owns q-heads [4c, 4c+4) and
kv-head c for both batches. After attention, per-head-pair AllToAlls (bf16)
switch head-sharding -> sequence-sharding (core c = batch c//4, seq block c%4
of 512); each core then normalizes and runs o_proj against the full Wo for its
512 rows.

All matmuls are bf16 with fp32 PSUM accumulation. Key structures:
  - qp[p] (bf16): per head-pair a [128, 4*1024] tile: batch0 in partitions
    0:64, batch1 in 64:128; columns tau*1024 + {h_even 512 | h_odd 512}.
  - kbT (bf16): [128, L]: batch0 kv-head in partitions 0:64, batch1 in 64:128,
    so transposed scores for the two batches run as concurrent row-tiled
    matmuls (row groups 0-1 and 2-3).
  - va (bf16): per key-block [128, 130]: cols 0:64 = v(b0), col 64 = ones,
    cols 65:129 = v(b1), col 129 = ones; the ones column makes the AV matmul
    emit the softmax denominator for free.
  - P (bf16, rolling 4-deep pool): per key-block [128, 2048] =
    [h0b0 | h1b0 | h0b1 | h1b1] x 512 queries.

The attention loop is head-pair-major so the first pair's AllToAll and the
first half of o_proj (parked in SBUF) overlap the second pair's compute; Wo is
prefetched to SBUF during attention.
"""

import os
import sys
import math

os.environ.setdefault("MYCRO_LOCAL_CACHE", "1")
for _p in ("/opt/trn_rl_repo",):
    if os.path.isdir(_p) and _p not in sys.path:
        sys.path.insert(0, _p)

import numpy as np
import ml_dtypes

import concourse.bass as bass
import concourse.bacc as bacc
import concourse.mybir as mybir
import concourse.tile as tile
from concourse.bass_utils import run_bass_kernel_spmd
from concourse.masks import make_identity

F32 = mybir.dt.float32
BF16 = mybir.dt.bfloat16
Exp = mybir.ActivationFunctionType.Exp

D = 2048
L = 2048
DH = 64
B = 2
NCORES = 8
NH_L = 4
QF = NH_L * DH
LC1 = 512
NLC1 = L // LC1
LT = 512
NT = L // LT
NB = L // 128
SH = 2 * (DH + 1)  # 130 rows per A2A shard
SCALE = 1.0 / math.sqrt(DH)

_CACHE = {}


def _build_nc():
    nc = bacc.Bacc(
        "TRN2",
        target_bir_lowering=False,
        debug=False,
        enable_asserts=False,
        num_devices=NCORES,
    )
    xh0 = nc.dram_tensor("xh0", [NLC1 * 128, 16 * LC1], BF16, kind="ExternalInput")
    xh1 = nc.dram_tensor("xh1", [NLC1 * 128, 16 * LC1], BF16, kind="ExternalInput")
    wqh = nc.dram_tensor("wqh", [128, 16 * QF], BF16, kind="ExternalInput")
    wkvh = nc.dram_tensor("wkvh", [128, 16 * 256], BF16, kind="ExternalInput")
    woh = nc.dram_tensor("woh", [D, D], BF16, kind="ExternalInput")
    y = nc.dram_tensor("y", [LT, D], F32, kind="ExternalOutput")

    with tile.TileContext(nc) as tc:
        with tc.tile_pool(name="dram", bufs=1, space="DRAM") as dram:
            bins = [
                dram.tile([NCORES * SH, LT], BF16, name=f"bin{hp}") for hp in range(2)
            ]
            bouts = [
                dram.tile([NCORES * SH, LT], BF16, name=f"bout{hp}") for hp in range(2)
            ]
            rdram = dram.tile([32, LT], BF16, name="rdram")

            with tc.tile_pool(name="const", bufs=1) as const:
                ident = const.tile([128, 128], BF16, name="ident")
                make_identity(nc, ident)

                with tc.tile_pool(name="pers", bufs=1) as pers:
                    qp = [
                        pers.tile([128, NT * 1024], BF16, name=f"qp{p}")
                        for p in range(2)
                    ]
                    kbT = pers.tile([128, L], BF16, name="kbT")
                    vaug = pers.tile([128, NB * 130], BF16, name="vaug")
                    va = vaug.rearrange("p (b c) -> p b c", c=130)
                    nc.gpsimd.memset(va[:, :, 64:65], 1.0)
                    nc.gpsimd.memset(va[:, :, 129:130], 1.0)

                    _phase1_qkv(nc, tc, xh0, xh1, wqh, wkvh, qp, kbT, va, ident)
                    with tc.tile_pool(name="wo", bufs=1) as wop:
                        wo_sb = wop.tile([128, 16 * D], BF16, name="wo_sb")
                        nc.gpsimd.dma_start(
                            wo_sb.rearrange("p (k d) -> p k d", d=D),
                            woh.rearrange("(k p) d -> p k d", p=128),
                        )
                        with tc.tile_pool(name="anp", bufs=1) as anp:
                            inner = (
                                tc.tile_pool(name="p2s", bufs=1, space="PSUM"),
                                tc.tile_pool(name="p2o", bufs=1, space="PSUM"),
                                tc.tile_pool(name="pbuf", bufs=4),
                                tc.tile_pool(name="stg", bufs=3),
                            )
                            scp = inner[0].__enter__()
                            ovp = inner[1].__enter__()
                            pbp = inner[2].__enter__()
                            stp = inner[3].__enter__()
                            pools = (scp, ovp, pbp, stp)
                            dvs = {}
                            ans = {}

                            def _prep_bcast(half):
                                """Denominator reciprocal + broadcast loads for
                                one half's chunks. Emitted mid-attention (for
                                half 0) so the chain executes while the other
                                pair computes, instead of gating o_proj."""
                                bo = bouts[half]
                                dall = anp.tile(
                                    [16, LT], BF16, name="dall", tag=f"dall{half}"
                                )
                                nc.gpsimd.dma_start(
                                    dall[:, :],
                                    bo.rearrange("(i r) c -> i r c", r=65)[:, 64, :],
                                )
                                rall = anp.tile(
                                    [16, LT], F32, name="rall", tag=f"rall{half}"
                                )
                                nc.vector.reciprocal(rall[:, :], dall[:, :])
                                rbf = anp.tile(
                                    [16, LT], BF16, name="rbf", tag=f"rbf{half}"
                                )
                                nc.vector.tensor_copy(rbf[:, :], rall[:, :])
                                nc.sync.dma_start(
                                    rdram[16 * half : 16 * (half + 1), :], rbf[:, :]
                                )
                                for c in range(NCORES):
                                    k = 2 * c + half
                                    base = SH * c
                                    au = anp.tile(
                                        [128, LT], BF16, name=f"au{k}", tag=f"au{k}"
                                    )
                                    nc.gpsimd.dma_start(
                                        au[0:64, :], bo[base : base + 64, :]
                                    )
                                    nc.gpsimd.dma_start(
                                        au[64:128, :], bo[base + 65 : base + 129, :]
                                    )
                                    dv = anp.tile(
                                        [128, LT], BF16, name=f"dv{k}", tag=f"dv{k}"
                                    )
                                    r0 = 16 * half + 2 * c
                                    nc.sync.dma_start(
                                        dv[0:64, :],
                                        rdram[r0 : r0 + 1, :].partition_broadcast(64),
                                    )
                                    nc.sync.dma_start(
                                        dv[64:128, :],
                                        rdram[r0 + 1 : r0 + 2, :].partition_broadcast(64),
                                    )
                                    dvs[k] = (au, dv)

                            def _mul_half(half):
                                for c in range(NCORES):
                                    k = 2 * c + half
                                    au, dv = dvs[k]
                                    an = anp.tile(
                                        [128, LT], BF16, name=f"an{k}", tag=f"an{k}"
                                    )
                                    nc.vector.tensor_mul(an[:, :], au[:, :], dv[:, :])
                                    ans[k] = an

                            for hp in range(2):
                                epi = (lambda: _prep_bcast(0)) if hp == 1 else None
                                _attn_pair(
                                    nc, tc, pools, qp, kbT, va, bins[hp], hp, epi
                                )
                                nc.gpsimd.collective_compute(
                                    "AllToAll",
                                    mybir.AluOpType.bypass,
                                    ins=[bins[hp].opt()],
                                    outs=[bouts[hp].opt()],
                                    replica_groups=[list(range(NCORES))],
                                )
                                if hp == 1:
                                    _mul_half(0)
                            _prep_bcast(1)
                            _mul_half(1)
                            for cm in reversed(inner):
                                cm.__exit__(None, None, None)
                            _phase4_oproj(nc, tc, ans, wo_sb, y)
    nc.finalize()
    return nc


def _phase1_qkv(nc, tc, xh0, xh1, wqh, wkvh, qp, kbT, va, ident):
    with (
        tc.tile_pool(name="w1", bufs=1) as wpool,
        tc.tile_pool(name="xc", bufs=2) as xpool,
        tc.tile_pool(name="vt", bufs=2) as vtpool,
        tc.tile_pool(name="p1q", bufs=1, space="PSUM") as p1q,
        tc.tile_pool(name="p1kv", bufs=1, space="PSUM") as p1kv,
    ):
        wq_sb = wpool.tile([128, 16 * QF], BF16, name="wq_sb")
        wkv_sb = wpool.tile([128, 16 * 256], BF16, name="wkv_sb")
        nc.sync.dma_start(wq_sb[:, :], wqh[:, :])
        nc.sync.dma_start(wkv_sb[:, :], wkvh[:, :])

        for lc in range(NLC1):
            x0 = xpool.tile([128, 16 * LC1], BF16, name="x0", tag="x0")
            x1 = xpool.tile([128, 16 * LC1], BF16, name="x1", tag="x1")
            for part in range(4):
                cs = slice(part * 4 * LC1, (part + 1) * 4 * LC1)
                nc.gpsimd.dma_start(x0[:, cs], xh0[lc * 128 : (lc + 1) * 128, cs])
                nc.gpsimd.dma_start(x1[:, cs], xh1[lc * 128 : (lc + 1) * 128, cs])
            cols = slice(lc * LC1, (lc + 1) * LC1)

            aq = [
                [
                    p1q.tile([128, LC1], F32, name=f"aq{p}{b}", tag=f"aq{p}{b}")
                    for b in range(2)
                ]
                for p in range(2)
            ]
            akv = [
                p1kv.tile([128, LC1], F32, name=f"akv{b}", tag=f"akv{b}")
                for b in range(2)
            ]
            for fb in range(16):
                st = dict(start=(fb == 0), stop=(fb == 15))
                x0f = x0[:, fb * LC1 : (fb + 1) * LC1]
                x1f = x1[:, fb * LC1 : (fb + 1) * LC1]
                for p in range(2):
                    w = wq_sb[:, fb * QF + p * 128 : fb * QF + (p + 1) * 128]
                    nc.tensor.matmul(aq[p][0][:, :], w, x0f, **st)
                    nc.tensor.matmul(aq[p][1][:, :], w, x1f, **st)
                wkv0 = wkv_sb[:, fb * 256 : fb * 256 + 128]
                wkv1 = wkv_sb[:, fb * 256 + 128 : (fb + 1) * 256]
                nc.tensor.matmul(akv[0][:, :], wkv0, x0f, **st)
                nc.tensor.matmul(akv[1][:, :], wkv1, x1f, **st)

            for p in range(2):
                qc = lc * 1024
                nc.scalar.copy(qp[p][0:64, qc : qc + 512], aq[p][0][0:64, :])
                nc.scalar.copy(qp[p][64:128, qc : qc + 512], aq[p][1][0:64, :])
                nc.scalar.copy(
                    qp[p][64:128, qc + 512 : qc + 1024], aq[p][1][64:128, :]
                )
                tq = vtpool.tile([128, LC1], BF16, name="tq", tag=f"tq{p}")
                nc.scalar.copy(tq[64:128, :], aq[p][0][64:128, :])
                nc.sync.dma_start(qp[p][0:64, qc + 512 : qc + 1024], tq[64:128, :])
            nc.scalar.copy(kbT[0:64, cols], akv[0][0:64, :])
            nc.scalar.copy(kbT[64:128, cols], akv[1][64:128, :])
            vsb = vtpool.tile([128, LC1], BF16, name="vsb", tag="vsb")
            nc.scalar.copy(vsb[0:64, :], akv[1][0:64, :])
            nc.scalar.copy(vsb[64:128, :], akv[0][64:128, :])
            for s in range(LC1 // 128):
                beta = (lc * LC1) // 128 + s
                tp = p1kv.tile([128, 128], BF16, name="tp", tag="tp", bufs=2)
                nc.tensor.matmul(
                    tp[:, :],
                    vsb[:, s * 128 : (s + 1) * 128],
                    ident[:, :],
                    is_transpose=True,
                )
                nc.scalar.copy(va[:, beta, 0:64], tp[:, 64:128])
                nc.scalar.copy(va[:, beta, 65:129], tp[:, 0:64])


def _attn_pair(nc, tc, pools, qp, kbT, va, bin_, hp, epilogue=None):
    scp, ovp, pbp, stp = pools
    h0, h1 = 2 * hp, 2 * hp + 1
    for tau in range(NT):
        if epilogue is not None and tau == NT - 1:
            epilogue()
        nb = 4 * tau + 4
        oab = ovp.tile([65, 1024], F32, name="oab", tag="oab")
        obb = ovp.tile([65, 1024], F32, name="obb", tag="obb")
        for blk in range(nb):
            dj = blk - 4 * tau
            off = max(dj, 0) * 128
            kc = slice(blk * 128, (blk + 1) * 128)
            tq = tau * 1024
            Pb = pbp.tile([128, 2048], BF16, name="Pb", tag="Pb")
            sb0 = scp.tile([128, 1024], F32, name="sb0", tag="sb0")
            sb1 = scp.tile([128, 1024], F32, name="sb1", tag="sb1")
            for hh, base in ((0, 0), (1, 512)):
                nc.tensor.matmul(
                    sb0[:, base + off : base + 512],
                    kbT[0:64, kc],
                    qp[hp][0:64, tq + base + off : tq + base + 512],
                    skip_group_check=(hh > 0),
                )
                nc.tensor.matmul(
                    sb1[:, base + off : base + 512],
                    kbT[64:128, kc],
                    qp[hp][64:128, tq + base + off : tq + base + 512],
                    skip_group_check=True,
                )
            if off:
                for base in (0, 512, 1024, 1536):
                    nc.gpsimd.memset(Pb[:, base : base + off], 0.0)
                for base in (0, 512):
                    nc.scalar.activation(
                        Pb[:, base + off : base + 512], sb0[:, base + off : base + 512], Exp
                    )
                    nc.scalar.activation(
                        Pb[:, 1024 + base + off : 1024 + base + 512],
                        sb1[:, base + off : base + 512],
                        Exp,
                    )
            else:
                nc.scalar.activation(Pb[:, 0:1024], sb0[:, :], Exp)
                nc.scalar.activation(Pb[:, 1024:2048], sb1[:, :], Exp)
            if dj >= 0:
                for base in (0, 512, 1024, 1536):
                    dg = Pb[:, base + off : base + off + 128]
                    nc.gpsimd.affine_select(
                        out=dg,
                        in_=dg,
                        compare_op=mybir.AluOpType.is_ge,
                        fill=0.0,
                        base=0,
                        pattern=[[1, 128]],
                        channel_multiplier=-1,
                    )
            st = dict(start=(blk == 0), stop=(blk == nb - 1))
            nc.tensor.matmul(oab[:, 0:512], va[:, blk, 0:65], Pb[:, 0:512], **st)
            nc.tensor.matmul(
                oab[:, 512:1024], va[:, blk, 0:65], Pb[:, 512:1024],
                skip_group_check=True, **st,
            )
            nc.tensor.matmul(
                obb[:, 0:512], va[:, blk, 65:130], Pb[:, 1024:1536],
                skip_group_check=True, **st,
            )
            nc.tensor.matmul(
                obb[:, 512:1024], va[:, blk, 65:130], Pb[:, 1536:2048],
                skip_group_check=True, **st,
            )

        for bb, src in ((0, oab), (1, obb)):
            sh = SH * (4 * bb + tau)
            for hh, base in ((0, 0), (1, 512)):
                stg = stp.tile([64, LT], BF16, name="stg", tag=f"stg{bb}{hh}")
                nc.vector.tensor_copy(stg[:, :], src[0:64, base : base + 512])
                nc.sync.dma_start(
                    bin_[sh + 65 * hh : sh + 65 * hh + 64, :], stg[:, :]
                )
        dsg = stp.tile([128, 2048], BF16, name="dsg", tag="dsg")
        nc.vector.tensor_copy(dsg[64:65, 0:512], oab[64:65, 0:512])
        nc.vector.tensor_copy(dsg[64:65, 512:1024], oab[64:65, 512:1024])
        nc.vector.tensor_copy(dsg[64:65, 1024:1536], obb[64:65, 0:512])
        nc.vector.tensor_copy(dsg[64:65, 1536:2048], obb[64:65, 512:1024])
        for i, (bb, hh) in enumerate(((0, 0), (0, 1), (1, 0), (1, 1))):
            sh = SH * (4 * bb + tau)
            r = sh + 65 * hh + 64
            nc.sync.dma_start(
                bin_[r : r + 1, :], dsg[64:65, 512 * i : 512 * (i + 1)]
            )


def _phase4_oproj(nc, tc, ans, wo_sb, y):
    """o_proj for this core's 512 rows against the full Wo, in two k-passes
    (one per A2A): pass A parks in SBUF while pass B's collective lands."""
    with (
        tc.tile_pool(name="ysum", bufs=1) as ysp,
        tc.tile_pool(name="ysb", bufs=2) as yp,
        tc.tile_pool(name="p4y", bufs=2, space="PSUM") as eyp,
    ):
        for half in range(2):
            if half == 0:
                ysum = [
                    ysp.tile([128, D], F32, name=f"ysum{m}", tag=f"ysum{m}")
                    for m in range(4)
                ]
                for m in range(4):
                    yps = eyp.tile([128, D], F32, name="yps", tag="yps")
                    for ki in range(NCORES):
                        k = 2 * ki
                        st = dict(start=(ki == 0), stop=(ki == NCORES - 1))
                        for q in range(4):
                            nc.tensor.matmul(
                                yps[:, q * 512 : (q + 1) * 512],
                                ans[k][:, m * 128 : (m + 1) * 128],
                                wo_sb[:, k * D + q * 512 : k * D + (q + 1) * 512],
                                skip_group_check=(q > 0),
                                **st,
                            )
                    nc.vector.tensor_copy(ysum[m][:, :], yps[:, :])
            else:
                for m in range(4):
                    yps = eyp.tile([128, D], F32, name="yps", tag="yps")
                    for ki in range(NCORES):
                        k = 2 * ki + 1
                        st = dict(start=(ki == 0), stop=(ki == NCORES - 1))
                        for q in range(4):
                            nc.tensor.matmul(
                                yps[:, q * 512 : (q + 1) * 512],
                                ans[k][:, m * 128 : (m + 1) * 128],
                                wo_sb[:, k * D + q * 512 : k * D + (q + 1) * 512],
                                skip_group_check=(q > 0),
                                **st,
                            )
                    ysb = yp.tile([128, D], F32, name="ysb", tag="ysb")
                    nc.vector.tensor_add(ysb[:, :], yps[:, :], ysum[m][:, :])
                    nc.sync.dma_start(y[m * 128 : (m + 1) * 128, :], ysb[:, :])


def _get_nc():
    if "nc" not in _CACHE:
        _CACHE["nc"] = _build_nc()
    return _CACHE["nc"]


LAST_EXEC_NS = None


def _prep_x(xb):
    xT = xb.T.astype(ml_dtypes.bfloat16)
    v = xT.reshape(16, 128, NLC1, LC1)
    v = v.transpose(2, 1, 0, 3)
    return np.ascontiguousarray(v.reshape(NLC1 * 128, 16 * LC1))


def kernel(x, Wq, Wk, Wv, Wo):
    global LAST_EXEC_NS
    x = np.asarray(x, dtype=np.float32)
    Wq = np.asarray(Wq, dtype=np.float32)
    Wk = np.asarray(Wk, dtype=np.float32)
    Wv = np.asarray(Wv, dtype=np.float32)
    Wo = np.asarray(Wo, dtype=np.float32)

    xh0 = _prep_x(x[0])
    xh1 = _prep_x(x[1])
    woh = np.ascontiguousarray(Wo.T.astype(ml_dtypes.bfloat16))

    in_maps = []
    for c in range(NCORES):
        wq_c = (SCALE * Wq[QF * c : QF * (c + 1), :]).astype(ml_dtypes.bfloat16)
        wqh = np.ascontiguousarray(
            wq_c.reshape(QF, 16, 128).transpose(2, 1, 0).reshape(128, 16 * QF)
        )
        wk_c = Wk[DH * c : DH * (c + 1), :].astype(ml_dtypes.bfloat16)
        wkh = wk_c.reshape(DH, 16, 128).transpose(2, 1, 0)
        wv_c = Wv[DH * c : DH * (c + 1), :].astype(ml_dtypes.bfloat16)
        wvh = wv_c.reshape(DH, 16, 128).transpose(2, 1, 0)
        wkvh = np.empty((128, 16, 256), dtype=ml_dtypes.bfloat16)
        wkvh[:, :, 0:64] = wkh
        wkvh[:, :, 64:128] = wvh
        wkvh[:, :, 128:192] = wvh
        wkvh[:, :, 192:256] = wkh
        wkvh = np.ascontiguousarray(wkvh.reshape(128, 16 * 256))
        in_maps.append(
            {"xh0": xh0, "xh1": xh1, "wqh": wqh, "wkvh": wkvh, "woh": woh}
        )

    nc = _get_nc()
    res = run_bass_kernel_spmd(nc, in_maps, core_ids=list(range(NCORES)))
    LAST_EXEC_NS = getattr(res, "exec_time_ns", None)

    out = np.empty((B, L, D), dtype=np.float32)
    for c in range(NCORES):
        b, g = divmod(c, 4)
        out[b, 512 * g : 512 * (g + 1), :] = res.results[c]["y"]
    return out


# revision 35
# speedup vs baseline: 1.0580x; 1.0580x over previous
"""Round-2 kernel (HW-verified: 471602 ns, rel err 6.6e-03). Kept as fallback.

GQA causal attention block (B=2, L=2048, d_model=2048, 32 Q heads / 8 KV heads)
on 8 TRN2 NeuronCores. See kernel.py for the evolved version.
"""

import os
import sys
import math

os.environ.setdefault("MYCRO_LOCAL_CACHE", "1")
for _p in ("/opt/trn_rl_repo",):
    if os.path.isdir(_p) and _p not in sys.path:
        sys.path.insert(0, _p)

import numpy as np
import ml_dtypes

import concourse.bass as bass
import concourse.bacc as bacc
import concourse.mybir as mybir
import concourse.tile as tile
from concourse.bass_utils import run_bass_kernel_spmd
from concourse.masks import make_identity

F32 = mybir.dt.float32
BF16 = mybir.dt.bfloat16
Exp = mybir.ActivationFunctionType.Exp

D = 2048
L = 2048
DH = 64
B = 2
NCORES = 8
NH_L = 4
QF = NH_L * DH
LC1 = 512
NLC1 = L // LC1
LT = 512
NT = L // LT
NB = L // 128
SH = 2 * (DH + 1)  # 130 rows per A2A shard
SCALE = 1.0 / math.sqrt(DH)

_CACHE = {}


def _build_nc():
    nc = bacc.Bacc(
        "TRN2",
        target_bir_lowering=False,
        debug=False,
        enable_asserts=False,
        num_devices=NCORES,
    )
    xh0 = nc.dram_tensor("xh0", [NLC1 * 128, 16 * LC1], BF16, kind="ExternalInput")
    xh1 = nc.dram_tensor("xh1", [NLC1 * 128, 16 * LC1], BF16, kind="ExternalInput")
    wqh = nc.dram_tensor("wqh", [128, 16 * QF], BF16, kind="ExternalInput")
    wkvh = nc.dram_tensor("wkvh", [128, 16 * 256], BF16, kind="ExternalInput")
    woh = nc.dram_tensor("woh", [D, D], BF16, kind="ExternalInput")
    y = nc.dram_tensor("y", [LT, D], F32, kind="ExternalOutput")

    with tile.TileContext(nc) as tc:
        with tc.tile_pool(name="dram", bufs=1, space="DRAM") as dram:
            bins = [
                dram.tile([NCORES * SH, LT], BF16, name=f"bin{hp}") for hp in range(2)
            ]
            bouts = [
                dram.tile([NCORES * SH, LT], BF16, name=f"bout{hp}") for hp in range(2)
            ]
            rdram = dram.tile([32, LT], BF16, name="rdram")

            with tc.tile_pool(name="const", bufs=1) as const:
                ident = const.tile([128, 128], BF16, name="ident")
                make_identity(nc, ident)

                with tc.tile_pool(name="pers", bufs=1) as pers:
                    qp = [
                        pers.tile([128, NT * 1024], BF16, name=f"qp{p}")
                        for p in range(2)
                    ]
                    kbT = pers.tile([128, L], BF16, name="kbT")
                    vaug = pers.tile([128, NB * 130], BF16, name="vaug")
                    va = vaug.rearrange("p (b c) -> p b c", c=130)
                    nc.gpsimd.memset(va[:, :, 64:65], 1.0)
                    nc.gpsimd.memset(va[:, :, 129:130], 1.0)

                    _phase1_qkv(nc, tc, xh0, xh1, wqh, wkvh, qp, kbT, va, ident)
                    with tc.tile_pool(name="wo", bufs=1) as wop:
                        wo_sb = wop.tile([128, 16 * D], BF16, name="wo_sb")
                        nc.gpsimd.dma_start(
                            wo_sb.rearrange("p (k d) -> p k d", d=D),
                            woh.rearrange("(k p) d -> p k d", p=128),
                        )
                        with (
                            tc.tile_pool(name="p2s", bufs=1, space="PSUM") as scp,
                            tc.tile_pool(name="p2o", bufs=1, space="PSUM") as ovp,
                            tc.tile_pool(name="pbuf", bufs=4) as pbp,
                            tc.tile_pool(name="stg", bufs=3) as stp,
                        ):
                            pools = (scp, ovp, pbp, stp)
                            for hp in range(2):
                                _attn_pair(nc, tc, pools, qp, kbT, va, bins[hp], hp)
                                nc.gpsimd.collective_compute(
                                    "AllToAll",
                                    mybir.AluOpType.bypass,
                                    ins=[bins[hp].opt()],
                                    outs=[bouts[hp].opt()],
                                    replica_groups=[list(range(NCORES))],
                                )
                        _phase4_oproj(nc, tc, bouts, rdram, wo_sb, y)
    nc.finalize()
    return nc


def _phase1_qkv(nc, tc, xh0, xh1, wqh, wkvh, qp, kbT, va, ident):
    with (
        tc.tile_pool(name="w1", bufs=1) as wpool,
        tc.tile_pool(name="xc", bufs=2) as xpool,
        tc.tile_pool(name="vt", bufs=2) as vtpool,
        tc.tile_pool(name="p1q", bufs=1, space="PSUM") as p1q,
        tc.tile_pool(name="p1kv", bufs=1, space="PSUM") as p1kv,
    ):
        wq_sb = wpool.tile([128, 16 * QF], BF16, name="wq_sb")
        wkv_sb = wpool.tile([128, 16 * 256], BF16, name="wkv_sb")
        nc.sync.dma_start(wq_sb[:, :], wqh[:, :])
        nc.sync.dma_start(wkv_sb[:, :], wkvh[:, :])

        for lc in range(NLC1):
            x0 = xpool.tile([128, 16 * LC1], BF16, name="x0", tag="x0")
            x1 = xpool.tile([128, 16 * LC1], BF16, name="x1", tag="x1")
            for part in range(4):
                cs = slice(part * 4 * LC1, (part + 1) * 4 * LC1)
                nc.gpsimd.dma_start(x0[:, cs], xh0[lc * 128 : (lc + 1) * 128, cs])
                nc.gpsimd.dma_start(x1[:, cs], xh1[lc * 128 : (lc + 1) * 128, cs])
            cols = slice(lc * LC1, (lc + 1) * LC1)

            aq = [
                [
                    p1q.tile([128, LC1], F32, name=f"aq{p}{b}", tag=f"aq{p}{b}")
                    for b in range(2)
                ]
                for p in range(2)
            ]
            akv = [
                p1kv.tile([128, LC1], F32, name=f"akv{b}", tag=f"akv{b}")
                for b in range(2)
            ]
            for fb in range(16):
                st = dict(start=(fb == 0), stop=(fb == 15))
                x0f = x0[:, fb * LC1 : (fb + 1) * LC1]
                x1f = x1[:, fb * LC1 : (fb + 1) * LC1]
                for p in range(2):
                    w = wq_sb[:, fb * QF + p * 128 : fb * QF + (p + 1) * 128]
                    nc.tensor.matmul(aq[p][0][:, :], w, x0f, **st)
                    nc.tensor.matmul(aq[p][1][:, :], w, x1f, **st)
                wkv0 = wkv_sb[:, fb * 256 : fb * 256 + 128]
                wkv1 = wkv_sb[:, fb * 256 + 128 : (fb + 1) * 256]
                nc.tensor.matmul(akv[0][:, :], wkv0, x0f, **st)
                nc.tensor.matmul(akv[1][:, :], wkv1, x1f, **st)

            for p in range(2):
                qc = lc * 1024
                nc.scalar.copy(qp[p][0:64, qc : qc + 512], aq[p][0][0:64, :])
                nc.scalar.copy(qp[p][64:128, qc : qc + 512], aq[p][1][0:64, :])
                nc.scalar.copy(
                    qp[p][64:128, qc + 512 : qc + 1024], aq[p][1][64:128, :]
                )
                tq = vtpool.tile([128, LC1], BF16, name="tq", tag=f"tq{p}")
                nc.scalar.copy(tq[64:128, :], aq[p][0][64:128, :])
                nc.sync.dma_start(qp[p][0:64, qc + 512 : qc + 1024], tq[64:128, :])
            nc.scalar.copy(kbT[0:64, cols], akv[0][0:64, :])
            nc.scalar.copy(kbT[64:128, cols], akv[1][64:128, :])
            vsb = vtpool.tile([128, LC1], BF16, name="vsb", tag="vsb")
            nc.scalar.copy(vsb[0:64, :], akv[1][0:64, :])
            nc.scalar.copy(vsb[64:128, :], akv[0][64:128, :])
            for s in range(LC1 // 128):
                beta = (lc * LC1) // 128 + s
                tp = p1kv.tile([128, 128], BF16, name="tp", tag="tp", bufs=2)
                nc.tensor.matmul(
                    tp[:, :],
                    vsb[:, s * 128 : (s + 1) * 128],
                    ident[:, :],
                    is_transpose=True,
                )
                nc.scalar.copy(va[:, beta, 0:64], tp[:, 64:128])
                nc.scalar.copy(va[:, beta, 65:129], tp[:, 0:64])


def _attn_pair(nc, tc, pools, qp, kbT, va, bin_, hp):
    scp, ovp, pbp, stp = pools
    h0, h1 = 2 * hp, 2 * hp + 1
    for tau in range(NT):
        nb = 4 * tau + 4
        oab = ovp.tile([65, 1024], F32, name="oab", tag="oab")
        obb = ovp.tile([65, 1024], F32, name="obb", tag="obb")
        for blk in range(nb):
            dj = blk - 4 * tau
            off = max(dj, 0) * 128
            kc = slice(blk * 128, (blk + 1) * 128)
            tq = tau * 1024
            Pb = pbp.tile([128, 2048], BF16, name="Pb", tag="Pb")
            sb0 = scp.tile([128, 1024], F32, name="sb0", tag="sb0")
            sb1 = scp.tile([128, 1024], F32, name="sb1", tag="sb1")
            for hh, base in ((0, 0), (1, 512)):
                nc.tensor.matmul(
                    sb0[:, base + off : base + 512],
                    kbT[0:64, kc],
                    qp[hp][0:64, tq + base + off : tq + base + 512],
                    skip_group_check=(hh > 0),
                )
                nc.tensor.matmul(
                    sb1[:, base + off : base + 512],
                    kbT[64:128, kc],
                    qp[hp][64:128, tq + base + off : tq + base + 512],
                    skip_group_check=True,
                )
            if off:
                for base in (0, 512, 1024, 1536):
                    nc.gpsimd.memset(Pb[:, base : base + off], 0.0)
                for base in (0, 512):
                    nc.scalar.activation(
                        Pb[:, base + off : base + 512], sb0[:, base + off : base + 512], Exp
                    )
                    nc.scalar.activation(
                        Pb[:, 1024 + base + off : 1024 + base + 512],
                        sb1[:, base + off : base + 512],
                        Exp,
                    )
            else:
                nc.scalar.activation(Pb[:, 0:1024], sb0[:, :], Exp)
                nc.scalar.activation(Pb[:, 1024:2048], sb1[:, :], Exp)
            if dj >= 0:
                for base in (0, 512, 1024, 1536):
                    dg = Pb[:, base + off : base + off + 128]
                    nc.gpsimd.affine_select(
                        out=dg,
                        in_=dg,
                        compare_op=mybir.AluOpType.is_ge,
                        fill=0.0,
                        base=0,
                        pattern=[[1, 128]],
                        channel_multiplier=-1,
                    )
            st = dict(start=(blk == 0), stop=(blk == nb - 1))
            nc.tensor.matmul(oab[:, 0:512], va[:, blk, 0:65], Pb[:, 0:512], **st)
            nc.tensor.matmul(
                oab[:, 512:1024], va[:, blk, 0:65], Pb[:, 512:1024],
                skip_group_check=True, **st,
            )
            nc.tensor.matmul(
                obb[:, 0:512], va[:, blk, 65:130], Pb[:, 1024:1536],
                skip_group_check=True, **st,
            )
            nc.tensor.matmul(
                obb[:, 512:1024], va[:, blk, 65:130], Pb[:, 1536:2048],
                skip_group_check=True, **st,
            )

        for bb, src in ((0, oab), (1, obb)):
            sh = SH * (4 * bb + tau)
            for hh, base in ((0, 0), (1, 512)):
                stg = stp.tile([64, LT], BF16, name="stg", tag=f"stg{bb}{hh}")
                nc.vector.tensor_copy(stg[:, :], src[0:64, base : base + 512])
                nc.sync.dma_start(
                    bin_[sh + 65 * hh : sh + 65 * hh + 64, :], stg[:, :]
                )
        dsg = stp.tile([128, 2048], BF16, name="dsg", tag="dsg")
        nc.vector.tensor_copy(dsg[64:65, 0:512], oab[64:65, 0:512])
        nc.vector.tensor_copy(dsg[64:65, 512:1024], oab[64:65, 512:1024])
        nc.vector.tensor_copy(dsg[64:65, 1024:1536], obb[64:65, 0:512])
        nc.vector.tensor_copy(dsg[64:65, 1536:2048], obb[64:65, 512:1024])
        for i, (bb, hh) in enumerate(((0, 0), (0, 1), (1, 0), (1, 1))):
            sh = SH * (4 * bb + tau)
            r = sh + 65 * hh + 64
            nc.sync.dma_start(
                bin_[r : r + 1, :], dsg[64:65, 512 * i : 512 * (i + 1)]
            )


def _phase4_oproj(nc, tc, bouts, rdram, wo_sb, y):
    with (
        tc.tile_pool(name="an", bufs=1) as anp,
        tc.tile_pool(name="ysum", bufs=1) as ysp,
        tc.tile_pool(name="ysb", bufs=2) as yp,
        tc.tile_pool(name="p4y", bufs=2, space="PSUM") as eyp,
    ):
        ans = {}
        for half in range(2):
            bo = bouts[half]
            dall = anp.tile([16, LT], BF16, name="dall", tag=f"dall{half}")
            nc.sync.dma_start(
                dall[:, :], bo.rearrange("(i r) c -> i r c", r=65)[:, 64, :]
            )
            rall = anp.tile([16, LT], F32, name="rall", tag=f"rall{half}")
            nc.vector.reciprocal(rall[:, :], dall[:, :])
            rbf = anp.tile([16, LT], BF16, name="rbf", tag=f"rbf{half}")
            nc.vector.tensor_copy(rbf[:, :], rall[:, :])
            nc.sync.dma_start(rdram[16 * half : 16 * (half + 1), :], rbf[:, :])
            for c in range(NCORES):
                k = 2 * c + half
                au = anp.tile([128, LT], BF16, name=f"au{k}", tag=f"au{k}")
                base = SH * c
                nc.sync.dma_start(au[0:64, :], bo[base : base + 64, :])
                nc.sync.dma_start(
                    au[64:128, :], bo[base + 65 : base + 129, :]
                )
                dv = anp.tile([128, LT], BF16, name="dv", tag="dv", bufs=2)
                nc.sync.dma_start(
                    dv[0:64, :],
                    rdram[
                        16 * half + 2 * c : 16 * half + 2 * c + 1, :
                    ].partition_broadcast(64),
                )
                nc.sync.dma_start(
                    dv[64:128, :],
                    rdram[
                        16 * half + 2 * c + 1 : 16 * half + 2 * c + 2, :
                    ].partition_broadcast(64),
                )
                an = anp.tile([128, LT], BF16, name=f"an{k}", tag=f"an{k}")
                nc.vector.tensor_mul(an[:, :], au[:, :], dv[:, :])
                ans[k] = an

            if half == 0:
                ysum = [
                    ysp.tile([128, D], F32, name=f"ysum{m}", tag=f"ysum{m}")
                    for m in range(4)
                ]
                for m in range(4):
                    yps = eyp.tile([128, D], F32, name="yps", tag="yps")
                    for ki in range(NCORES):
                        k = 2 * ki
                        st = dict(start=(ki == 0), stop=(ki == NCORES - 1))
                        for q in range(4):
                            nc.tensor.matmul(
                                yps[:, q * 512 : (q + 1) * 512],
                                ans[k][:, m * 128 : (m + 1) * 128],
                                wo_sb[:, k * D + q * 512 : k * D + (q + 1) * 512],
                                skip_group_check=(q > 0),
                                **st,
                            )
                    nc.vector.tensor_copy(ysum[m][:, :], yps[:, :])
            else:
                for m in range(4):
                    yps = eyp.tile([128, D], F32, name="yps", tag="yps")
                    for ki in range(NCORES):
                        k = 2 * ki + 1
                        st = dict(start=(ki == 0), stop=(ki == NCORES - 1))
                        for q in range(4):
                            nc.tensor.matmul(
                                yps[:, q * 512 : (q + 1) * 512],
                                ans[k][:, m * 128 : (m + 1) * 128],
                                wo_sb[:, k * D + q * 512 : k * D + (q + 1) * 512],
                                skip_group_check=(q > 0),
                                **st,
                            )
                    ysb = yp.tile([128, D], F32, name="ysb", tag="ysb")
                    nc.vector.tensor_add(ysb[:, :], yps[:, :], ysum[m][:, :])
                    nc.sync.dma_start(y[m * 128 : (m + 1) * 128, :], ysb[:, :])


def _get_nc():
    if "nc" not in _CACHE:
        _CACHE["nc"] = _build_nc()
    return _CACHE["nc"]


LAST_EXEC_NS = None


def _prep_x(xb):
    xT = xb.T.astype(ml_dtypes.bfloat16)
    v = xT.reshape(16, 128, NLC1, LC1)
    v = v.transpose(2, 1, 0, 3)
    return np.ascontiguousarray(v.reshape(NLC1 * 128, 16 * LC1))


def kernel(x, Wq, Wk, Wv, Wo):
    global LAST_EXEC_NS
    x = np.asarray(x, dtype=np.float32)
    Wq = np.asarray(Wq, dtype=np.float32)
    Wk = np.asarray(Wk, dtype=np.float32)
    Wv = np.asarray(Wv, dtype=np.float32)
    Wo = np.asarray(Wo, dtype=np.float32)

    xh0 = _prep_x(x[0])
    xh1 = _prep_x(x[1])
    woh = np.ascontiguousarray(Wo.T.astype(ml_dtypes.bfloat16))

    in_maps = []
    for c in range(NCORES):
        wq_c = (SCALE * Wq[QF * c : QF * (c + 1), :]).astype(ml_dtypes.bfloat16)
        wqh = np.ascontiguousarray(
            wq_c.reshape(QF, 16, 128).transpose(2, 1, 0).reshape(128, 16 * QF)
        )
        wk_c = Wk[DH * c : DH * (c + 1), :].astype(ml_dtypes.bfloat16)
        wkh = wk_c.reshape(DH, 16, 128).transpose(2, 1, 0)
        wv_c = Wv[DH * c : DH * (c + 1), :].astype(ml_dtypes.bfloat16)
        wvh = wv_c.reshape(DH, 16, 128).transpose(2, 1, 0)
        wkvh = np.empty((128, 16, 256), dtype=ml_dtypes.bfloat16)
        wkvh[:, :, 0:64] = wkh
        wkvh[:, :, 64:128] = wvh
        wkvh[:, :, 128:192] = wvh
        wkvh[:, :, 192:256] = wkh
        wkvh = np.ascontiguousarray(wkvh.reshape(128, 16 * 256))
        in_maps.append(
            {"xh0": xh0, "xh1": xh1, "wqh": wqh, "wkvh": wkvh, "woh": woh}
        )

    nc = _get_nc()
    res = run_bass_kernel_spmd(nc, in_maps, core_ids=list(range(NCORES)))
    LAST_EXEC_NS = getattr(res, "exec_time_ns", None)

    out = np.empty((B, L, D), dtype=np.float32)
    for c in range(NCORES):
        b, g = divmod(c, 4)
        out[b, 512 * g : 512 * (g + 1), :] = res.results[c]["y"]
    return out
